# revision 4
# baseline (speedup 1.0000x reference)
"""Trainium2 Bass kernel for nn_EquivariantDipoleMoment.

Strategy (data-parallel over atoms, 8 cores):
  - Host shards s/v over the atom axis and transposes to feature-major
    layouts ([F, atoms]) so every DMA is wide and contiguous.
  - Each core runs the two PaiNN-style GatedEquivariant blocks plus the
    final Ws/Wvf projections for its 8192 atoms, producing per-atom
    metrics [vv_x, vv_y, vv_z, s_out] (a [4, 8192] tensor).
  - The per-molecule segment reductions (all linear in the per-atom
    metrics), the centre computation, and the final norm are tiny
    (O(N*4 + NB*3) flops) and are done on the host in float64.

On-chip layout: features on partitions, atoms on the free dim.  All
matmuls keep that orientation (lhsT = weight, rhs = activations).  The
atom range of a core is processed in two 4096-atom halves so SBUF fits
in fp32, and within a half each phase is batched so the ACT engine never
ping-pongs between the sqrt and silu table sets.
"""

import numpy as np

N = 65536
NB = 64
F = 128
NCORES = 8
M = N // NCORES  # atoms per core

_F32 = None  # set lazily (mybir.dt.float32)


def _emit(tc, outs, ins, m):
    """Emit the per-core program.  ins/outs are dicts of bass APs.

    m: atoms processed by this core (must be a multiple of 1024; the
    half/chunk structure scales down for small simulation shapes).
    """
    import concourse.mybir as mybir
    from concourse.mybir import ActivationFunctionType as AF
    from concourse.alu_op_type import AluOpType as ALU

    nc = tc.nc
    f32 = mybir.dt.float32

    C = min(1024, m // 2)          # chunk size (atoms)
    HALVES = 2
    MH = m // HALVES               # atoms per half
    NCH = MH // C                  # chunks per half
    NS = C // 512 if C >= 512 else 1   # 512-col matmul slices per chunk
    MS = C // 512 if C >= 512 else 1   # met subchunks
    MC = min(512, C)               # met subchunk cols

    from contextlib import ExitStack
    ctx = ExitStack()

    wpool = ctx.enter_context(tc.tile_pool(name="wpool", bufs=1))
    vthpool = ctx.enter_context(tc.tile_pool(name="vthpool", bufs=1))
    persist = ctx.enter_context(tc.tile_pool(name="persist", bufs=1))
    sxpool = ctx.enter_context(tc.tile_pool(name="sxpool", bufs=2))
    sqpool = ctx.enter_context(tc.tile_pool(name="sqpool", bufs=2))
    vn2pool = ctx.enter_context(tc.tile_pool(name="vn2pool", bufs=2))
    h1pool = ctx.enter_context(tc.tile_pool(name="h1pool", bufs=2))
    gatepool = ctx.enter_context(tc.tile_pool(name="gatepool", bufs=2))
    s2tpool = ctx.enter_context(tc.tile_pool(name="s2tpool", bufs=2))
    metsb = ctx.enter_context(tc.tile_pool(name="metsb", bufs=2))
    psum = ctx.enter_context(tc.tile_pool(name="psum", bufs=3, space="PSUM"))
    psumM = ctx.enter_context(tc.tile_pool(name="psumM", bufs=2, space="PSUM"))

    # ---- load weights & biases into SBUF once -------------------------
    wt = {}
    for name in (
        "Wu0", "Wv0", "Wu1", "Wv1",
        "W1a0", "W1b0", "W1a1", "W1b1",
        "W2a0", "W2b0", "W2a1", "W2b1",
    ):
        t = wpool.tile([128, 128], f32, name=f"w_{name}", tag=f"w_{name}")
        nc.sync.dma_start(t[:], ins[name])
        wt[name] = t
    for name in ("WvfC0", "WvfC1", "WvfC2", "WsC3"):
        t = wpool.tile([128, 4], f32, name=f"w_{name}", tag=f"w_{name}")
        nc.sync.dma_start(t[:], ins[name])
        wt[name] = t
    for name in ("b1_0", "b1_1", "b2a0", "b2b0", "b2a1", "b2b1"):
        t = wpool.tile([128, 1], f32, name=f"b_{name}", tag=f"b_{name}")
        nc.sync.dma_start(t[:], ins[name])
        wt[name] = t

    Wu = (wt["Wu0"], wt["Wu1"])
    Wv = (wt["Wv0"], wt["Wv1"])
    W1a = (wt["W1a0"], wt["W1a1"])
    W1b = (wt["W1b0"], wt["W1b1"])
    W2a = (wt["W2a0"], wt["W2a1"])
    W2b = (wt["W2b0"], wt["W2b1"])
    B1 = (wt["b1_0"], wt["b1_1"])
    B2A = (wt["b2a0"], wt["b2a1"])
    B2B = (wt["b2b0"], wt["b2b1"])
    WVFC = (wt["WvfC0"], wt["WvfC1"], wt["WvfC2"])

    sT = ins["sT"]          # [128, m] DRAM
    vT = ins["vT"]          # [128, 3, m] DRAM
    met_out = outs["met"]   # [4, m] DRAM

    for H in range(HALVES):
        h0 = H * MH
        # v chunk-sliced load for this half (feature-major, d-major)
        vTH = vthpool.tile([128, 3, MH], f32, name=f"vTH{H}", tag="vTH")
        for i in range(NCH):
            sl = slice(i * C, (i + 1) * C)
            nc.sync.dma_start(vTH[:, :, sl], vT[:, :, h0 + i * C:h0 + (i + 1) * C])

        s2H = None
        vnext = None
        for blk in range(2):
            # ---------------- phase A: vn = ||v @ Wu|| (sqrt table set) ----
            vn = persist.tile([128, MH], f32, name=f"vn{H}{blk}", tag="vn")
            for i in range(NCH):
                sl = slice(i * C, (i + 1) * C)
                vsrc = vTH if blk == 0 else vnext
                acc = vn2pool.tile([128, C], f32, name=f"acc{H}{blk}{i}", tag="acc")
                for d in range(3):
                    v1 = psum.tile([128, C], f32, name=f"v1_{H}{blk}{i}{d}", tag="mm")
                    for s5 in range(NS):
                        cs = slice(s5 * 512, min((s5 + 1) * 512, C))
                        nc.tensor.matmul(
                            v1[:, cs], Wu[blk][:], vsrc[:, d, sl][:, cs],
                            start=True, stop=True,
                        )
                    if d == 0:
                        nc.scalar.square(acc[:], v1[:])
                    else:
                        sq = sqpool.tile([128, C], f32, name=f"sq{H}{blk}{i}{d}", tag="sq")
                        nc.scalar.square(sq[:], v1[:])
                        nc.gpsimd.tensor_tensor(acc[:], acc[:], sq[:], ALU.add)
                nc.scalar.sqrt(vn[:, sl], acc[:])

            # ---------------- phase BCD (silu table set) -------------------
            if blk == 0:
                s2H = persist.tile([128, MH], f32, name=f"s2H{H}", tag="s2")
                vnext = persist.tile([128, 3, MH], f32, name=f"vnext{H}", tag="vnext")
            for i in range(NCH):
                sl = slice(i * C, (i + 1) * C)
                # s-path input chunk
                if blk == 0:
                    s_in = sxpool.tile([128, C], f32, name=f"sin{H}{i}", tag="sx")
                    nc.sync.dma_start(s_in[:], sT[:, h0 + i * C:h0 + (i + 1) * C])
                else:
                    s_in = s2H[:, sl]
                # B: h1 = silu(W1a.s + W1b.vn + b1)
                h1 = psum.tile([128, C], f32, name=f"h1_{H}{blk}{i}", tag="mm")
                for s5 in range(NS):
                    cs = slice(s5 * 512, min((s5 + 1) * 512, C))
                    nc.tensor.matmul(h1[:, cs], W1a[blk][:], s_in[:, cs],
                                     start=True, stop=False)
                    nc.tensor.matmul(h1[:, cs], W1b[blk][:], vn[:, sl][:, cs],
                                     start=False, stop=True)
                h1s = h1pool.tile([128, C], f32, name=f"h1s{H}{blk}{i}", tag="h1s")
                nc.scalar.activation(h1s[:], h1[:], AF.Silu, bias=B1[blk][:])
                # C: s2 / gate
                s2p = psum.tile([128, C], f32, name=f"s2p{H}{blk}{i}", tag="mm")
                gatep = psum.tile([128, C], f32, name=f"gp{H}{blk}{i}", tag="mm")
                for s5 in range(NS):
                    cs = slice(s5 * 512, min((s5 + 1) * 512, C))
                    nc.tensor.matmul(s2p[:, cs], W2a[blk][:], h1s[:, cs],
                                     start=True, stop=True)
                for s5 in range(NS):
                    cs = slice(s5 * 512, min((s5 + 1) * 512, C))
                    nc.tensor.matmul(gatep[:, cs], W2b[blk][:], h1s[:, cs],
                                     start=True, stop=True)
                gate = gatepool.tile([128, C], f32, name=f"gate{H}{blk}{i}", tag="gate")
                nc.vector.tensor_scalar_add(gate[:], gatep[:], B2B[blk][:])
                if blk == 0:
                    nc.vector.tensor_scalar_add(s2H[:, sl], s2p[:], B2A[0][:])
                    s2t = None
                else:
                    s2t = s2tpool.tile([128, C], f32, name=f"s2t{H}{i}", tag="s2t")
                    nc.vector.tensor_scalar_add(s2t[:], s2p[:], B2A[1][:])
                # D: v2 = (v @ Wv) * gate
                vsrc = vTH if blk == 0 else vnext
                if blk == 1:
                    metp = [
                        psumM.tile([4, MC], f32, name=f"metp{H}{i}{s5}", tag="met")
                        for s5 in range(MS)
                    ]
                for d in range(3):
                    v2p = psum.tile([128, C], f32, name=f"v2p{H}{blk}{i}{d}", tag="mm")
                    for s5 in range(NS):
                        cs = slice(s5 * 512, min((s5 + 1) * 512, C))
                        nc.tensor.matmul(v2p[:, cs], Wv[blk][:], vsrc[:, d, sl][:, cs],
                                         start=True, stop=True)
                    if blk == 0:
                        nc.vector.tensor_tensor(vnext[:, d, sl], v2p[:], gate[:], ALU.mult)
                    else:
                        vx = sxpool.tile([128, C], f32, name=f"vx{H}{i}{d}", tag="sx")
                        nc.vector.tensor_tensor(vx[:], v2p[:], gate[:], ALU.mult)
                        for s5 in range(MS):
                            cs = slice(s5 * MC, (s5 + 1) * MC)
                            nc.tensor.matmul(metp[s5][:], WVFC[d][:], vx[:, cs],
                                             start=(d == 0), stop=False)
                if blk == 1:
                    for s5 in range(MS):
                        cs = slice(s5 * MC, (s5 + 1) * MC)
                        nc.tensor.matmul(metp[s5][:], wt["WsC3"][:], s2t[:, cs],
                                         start=False, stop=True)
                        mets = metsb.tile([4, MC], f32, name=f"mets{H}{i}{s5}", tag="met_sb")
                        nc.scalar.copy(mets[:], metp[s5][:])
                        nc.sync.dma_start(
                            met_out[:, h0 + i * C + s5 * MC:h0 + i * C + (s5 + 1) * MC],
                            mets[:],
                        )
    ctx.close()


def _prep_weights(inputs):
    """Host-side: split / pad the small weights into lhsT-ready arrays."""
    g = lambda k: np.ascontiguousarray(np.asarray(inputs[k], dtype=np.float32))
    out = {}
    for blk in range(2):
        out[f"Wu{blk}"] = g(f"Wu{blk}")
        out[f"Wv{blk}"] = g(f"Wv{blk}")
        W1 = g(f"W1_{blk}")
        out[f"W1a{blk}"] = np.ascontiguousarray(W1[:128, :])
        out[f"W1b{blk}"] = np.ascontiguousarray(W1[128:, :])
        W2 = g(f"W2_{blk}")
        out[f"W2a{blk}"] = np.ascontiguousarray(W2[:, :128])
        out[f"W2b{blk}"] = np.ascontiguousarray(W2[:, 128:])
        b1 = g(f"b1_{blk}").reshape(128, 1)
        b2 = g(f"b2_{blk}")
        out[f"b1_{blk}"] = np.ascontiguousarray(b1)
        out[f"b2a{blk}"] = np.ascontiguousarray(b2[:128].reshape(128, 1))
        out[f"b2b{blk}"] = np.ascontiguousarray(b2[128:].reshape(128, 1))
    Wvf = g("Wvf").reshape(128)
    Ws = g("Ws").reshape(128)
    for d in range(3):
        c = np.zeros((128, 4), np.float32)
        c[:, d] = Wvf
        out[f"WvfC{d}"] = c
    c = np.zeros((128, 4), np.float32)
    c[:, 3] = Ws
    out["WsC3"] = c
    return out


_CACHE = {}


def _build_program(m):
    if m in _CACHE:
        return _CACHE[m]
    from contextlib import ExitStack
    import concourse.bacc as bacc
    import concourse.mybir as mybir
    from concourse.tile import TileContext

    f32 = mybir.dt.float32
    nc = bacc.Bacc("TRN2", target_bir_lowering=False, debug=False, num_devices=1)
    ins = {}
    ins["sT"] = nc.dram_tensor("sT", [128, m], f32, kind="ExternalInput").ap()
    ins["vT"] = nc.dram_tensor("vT", [128, 3, m], f32, kind="ExternalInput").ap()
    for name in ("Wu0", "Wv0", "Wu1", "Wv1", "W1a0", "W1b0", "W1a1", "W1b1",
                 "W2a0", "W2b0", "W2a1", "W2b1"):
        ins[name] = nc.dram_tensor(name, [128, 128], f32, kind="ExternalInput").ap()
    for name in ("WvfC0", "WvfC1", "WvfC2", "WsC3"):
        ins[name] = nc.dram_tensor(name, [128, 4], f32, kind="ExternalInput").ap()
    for name in ("b1_0", "b1_1", "b2a0", "b2b0", "b2a1", "b2b1"):
        ins[name] = nc.dram_tensor(name, [128, 1], f32, kind="ExternalInput").ap()
    outs = {"met": nc.dram_tensor("met", [4, m], f32, kind="ExternalOutput").ap()}

    with TileContext(nc) as tc:
        _emit(tc, outs, ins, m)
    nc.finalize()
    _CACHE[m] = nc
    return nc


def _host_combine(met_full, z0, r0, mask2d):
    """met_full: [4, N] per-atom metrics; final reduction in float64."""
    vv = met_full[:3, :].T.astype(np.float64)          # [N, 3]
    s_out = met_full[3, :].astype(np.float64)          # [N]
    m64 = mask2d.astype(np.float64)                    # [NB, N]
    z64 = z0.astype(np.float64)
    r64 = r0.astype(np.float64)

    A = m64 @ vv                                       # [NB, 3]
    B = m64 @ (s_out[:, None] * r64)                   # [NB, 3]
    Nz = m64 @ (z64[:, None] * r64)                    # [NB, 3]
    Dz = m64 @ z64                                     # [NB]
    S = s_out.sum()
    centre = Nz / Dz[:, None]
    mu = A + B - centre * S
    return np.linalg.norm(mu, axis=-1, keepdims=True).astype(np.float32)  # [NB,1]


def kernel(**inputs):
    from concourse.bass_utils import run_bass_kernel_spmd

    z = np.asarray(inputs["z"], dtype=np.float32)
    s = np.asarray(inputs["s"], dtype=np.float32)
    v = np.asarray(inputs["v"], dtype=np.float32)
    r = np.asarray(inputs["r"], dtype=np.float32)
    batch_mask = np.asarray(inputs["batch_mask"], dtype=np.float32)

    w = _prep_weights(inputs)

    sT_full = np.ascontiguousarray(s[0].T)                     # [128, N]
    vT_full = np.ascontiguousarray(v[0].transpose(2, 1, 0))    # [128, 3, N]

    in_maps = []
    for c in range(NCORES):
        a, b = c * M, (c + 1) * M
        im = dict(w)
        im["sT"] = np.ascontiguousarray(sT_full[:, a:b])
        im["vT"] = np.ascontiguousarray(vT_full[:, :, a:b])
        in_maps.append(im)

    nc = _build_program(M)
    res = run_bass_kernel_spmd(nc, in_maps, core_ids=list(range(NCORES)))
    met_full = np.concatenate([res.results[c]["met"] for c in range(NCORES)], axis=1)

    return _host_combine(met_full, z[0], r[0], batch_mask[:, :, 0])


# revision 9
# speedup vs baseline: 12.7341x; 12.7341x over previous
"""Trainium2 Bass kernel for nn_EquivariantDipoleMoment.

Strategy (data-parallel over atoms, 8 cores):
  - Host shards s/v over the atom axis and transposes to feature-major
    layouts ([F, atoms]) so every DMA is wide and contiguous.
  - Each core runs the two PaiNN-style GatedEquivariant blocks plus the
    final Ws/Wvf projections for its 8192 atoms, producing per-atom
    metrics [vv_x, vv_y, vv_z, s_out] (a [4, 8192] tensor).
  - The per-molecule segment reductions (all linear in the per-atom
    metrics), the centre computation, and the final norm are tiny
    (O(N*4 + NB*3) flops) and are done on the host in float64.

On-chip layout: features on partitions, atoms on the free dim.  All
matmuls keep that orientation (lhsT = weight, rhs = activations).  The
atom range of a core is processed in two 4096-atom halves so SBUF fits
in fp32, and within a half each phase is batched so the ACT engine never
ping-pongs between the sqrt and silu table sets.
"""

import numpy as np

N = 65536
NB = 64
F = 128
NCORES = 8
M = N // NCORES  # atoms per core

_F32 = None  # set lazily (mybir.dt.float32)


def _emit(tc, outs, ins, m, reps=1):
    """Emit the per-core program.  ins/outs are dicts of bass APs.

    m: atoms processed by this core (must be a multiple of 1024; the
    half/chunk structure scales down for small simulation shapes).
    reps: emit the body multiple times (timing variant) reusing the same
    tile pools, so SBUF usage is unchanged.
    """
    import concourse.mybir as mybir
    from concourse.mybir import ActivationFunctionType as AF
    from concourse.alu_op_type import AluOpType as ALU

    nc = tc.nc
    f32 = mybir.dt.float32

    C = min(1024, m // 2)          # chunk size (atoms)
    HALVES = 2
    MH = m // HALVES               # atoms per half
    NCH = MH // C                  # chunks per half
    NS = C // 512 if C >= 512 else 1   # 512-col matmul slices per chunk
    MS = C // 512 if C >= 512 else 1   # met subchunks
    MC = min(512, C)               # met subchunk cols

    from contextlib import ExitStack
    ctx = ExitStack()

    wpool = ctx.enter_context(tc.tile_pool(name="wpool", bufs=1))
    vthpool = ctx.enter_context(tc.tile_pool(name="vthpool", bufs=1))
    persist = ctx.enter_context(tc.tile_pool(name="persist", bufs=1))
    sxpool = ctx.enter_context(tc.tile_pool(name="sxpool", bufs=2))
    sqpool = ctx.enter_context(tc.tile_pool(name="sqpool", bufs=2))
    vn2pool = ctx.enter_context(tc.tile_pool(name="vn2pool", bufs=2))
    h1pool = ctx.enter_context(tc.tile_pool(name="h1pool", bufs=2))
    gatepool = ctx.enter_context(tc.tile_pool(name="gatepool", bufs=2))
    s2tpool = ctx.enter_context(tc.tile_pool(name="s2tpool", bufs=2))
    metsb = ctx.enter_context(tc.tile_pool(name="metsb", bufs=2))
    psum = ctx.enter_context(tc.tile_pool(name="psum", bufs=3, space="PSUM"))
    psumM = ctx.enter_context(tc.tile_pool(name="psumM", bufs=2, space="PSUM"))

    # ---- load weights & biases into SBUF once -------------------------
    wt = {}
    for name in (
        "Wu0", "Wv0", "Wu1", "Wv1",
        "W1a0", "W1b0", "W1a1", "W1b1",
        "W2a0", "W2b0", "W2a1", "W2b1",
    ):
        t = wpool.tile([128, 128], f32, name=f"w_{name}", tag=f"w_{name}")
        nc.sync.dma_start(t[:], ins[name])
        wt[name] = t
    for name in ("WvfC0", "WvfC1", "WvfC2", "WsC3"):
        t = wpool.tile([128, 4], f32, name=f"w_{name}", tag=f"w_{name}")
        nc.sync.dma_start(t[:], ins[name])
        wt[name] = t
    for name in ("b1_0", "b1_1", "b2a0", "b2b0", "b2a1", "b2b1"):
        t = wpool.tile([128, 1], f32, name=f"b_{name}", tag=f"b_{name}")
        nc.sync.dma_start(t[:], ins[name])
        wt[name] = t

    Wu = (wt["Wu0"], wt["Wu1"])
    Wv = (wt["Wv0"], wt["Wv1"])
    W1a = (wt["W1a0"], wt["W1a1"])
    W1b = (wt["W1b0"], wt["W1b1"])
    W2a = (wt["W2a0"], wt["W2a1"])
    W2b = (wt["W2b0"], wt["W2b1"])
    B1 = (wt["b1_0"], wt["b1_1"])
    B2A = (wt["b2a0"], wt["b2a1"])
    B2B = (wt["b2b0"], wt["b2b1"])
    WVFC = (wt["WvfC0"], wt["WvfC1"], wt["WvfC2"])

    sT = ins["sT"]          # [128, m] DRAM
    vT = ins["vT"]          # [128, 3, m] DRAM
    met_out = outs["met"]   # [4, m] DRAM

    def _body(rep):
     for H in range(HALVES):
        h0 = H * MH
        # v chunk-sliced load for this half (feature-major, d-major)
        vTH = vthpool.tile([128, 3, MH], f32, name=f"vTH{H}", tag="vTH")
        for i in range(NCH):
            sl = slice(i * C, (i + 1) * C)
            nc.sync.dma_start(vTH[:, :, sl], vT[:, :, h0 + i * C:h0 + (i + 1) * C])

        s2H = None
        vnext = None
        for blk in range(2):
            # ---------------- phase A: vn = ||v @ Wu|| (sqrt table set) ----
            vn = persist.tile([128, MH], f32, name=f"vn{H}{blk}", tag="vn")
            for i in range(NCH):
                sl = slice(i * C, (i + 1) * C)
                vsrc = vTH if blk == 0 else vnext
                acc = vn2pool.tile([128, C], f32, name=f"acc{H}{blk}{i}", tag="acc")
                for d in range(3):
                    v1 = psum.tile([128, C], f32, name=f"v1_{H}{blk}{i}{d}", tag="mm")
                    for s5 in range(NS):
                        cs = slice(s5 * 512, min((s5 + 1) * 512, C))
                        nc.tensor.matmul(
                            v1[:, cs], Wu[blk][:], vsrc[:, d, sl][:, cs],
                            start=True, stop=True,
                        )
                    if d == 0:
                        nc.scalar.square(acc[:], v1[:])
                    else:
                        sq = sqpool.tile([128, C], f32, name=f"sq{H}{blk}{i}{d}", tag="sq")
                        nc.scalar.square(sq[:], v1[:])
                        nc.gpsimd.tensor_tensor(acc[:], acc[:], sq[:], ALU.add)
                nc.scalar.sqrt(vn[:, sl], acc[:])

            # ---------------- phase BCD (silu table set) -------------------
            if blk == 0:
                s2H = persist.tile([128, MH], f32, name=f"s2H{H}", tag="s2")
                vnext = persist.tile([128, 3, MH], f32, name=f"vnext{H}", tag="vnext")
            for i in range(NCH):
                sl = slice(i * C, (i + 1) * C)
                # s-path input chunk
                if blk == 0:
                    s_in = sxpool.tile([128, C], f32, name=f"sin{H}{i}", tag="sx")
                    nc.sync.dma_start(s_in[:], sT[:, h0 + i * C:h0 + (i + 1) * C])
                else:
                    s_in = s2H[:, sl]
                # B: h1 = silu(W1a.s + W1b.vn + b1)
                h1 = psum.tile([128, C], f32, name=f"h1_{H}{blk}{i}", tag="mm")
                for s5 in range(NS):
                    cs = slice(s5 * 512, min((s5 + 1) * 512, C))
                    nc.tensor.matmul(h1[:, cs], W1a[blk][:], s_in[:, cs],
                                     start=True, stop=False)
                    nc.tensor.matmul(h1[:, cs], W1b[blk][:], vn[:, sl][:, cs],
                                     start=False, stop=True)
                h1s = h1pool.tile([128, C], f32, name=f"h1s{H}{blk}{i}", tag="h1s")
                nc.scalar.activation(h1s[:], h1[:], AF.Silu, bias=B1[blk][:])
                # C: s2 / gate
                s2p = psum.tile([128, C], f32, name=f"s2p{H}{blk}{i}", tag="mm")
                gatep = psum.tile([128, C], f32, name=f"gp{H}{blk}{i}", tag="mm")
                for s5 in range(NS):
                    cs = slice(s5 * 512, min((s5 + 1) * 512, C))
                    nc.tensor.matmul(s2p[:, cs], W2a[blk][:], h1s[:, cs],
                                     start=True, stop=True)
                for s5 in range(NS):
                    cs = slice(s5 * 512, min((s5 + 1) * 512, C))
                    nc.tensor.matmul(gatep[:, cs], W2b[blk][:], h1s[:, cs],
                                     start=True, stop=True)
                gate = gatepool.tile([128, C], f32, name=f"gate{H}{blk}{i}", tag="gate")
                nc.vector.tensor_scalar_add(gate[:], gatep[:], B2B[blk][:])
                if blk == 0:
                    nc.vector.tensor_scalar_add(s2H[:, sl], s2p[:], B2A[0][:])
                    s2t = None
                else:
                    s2t = s2tpool.tile([128, C], f32, name=f"s2t{H}{i}", tag="s2t")
                    nc.vector.tensor_scalar_add(s2t[:], s2p[:], B2A[1][:])
                # D: v2 = (v @ Wv) * gate
                vsrc = vTH if blk == 0 else vnext
                if blk == 1:
                    metp = [
                        psumM.tile([4, MC], f32, name=f"metp{H}{i}{s5}", tag="met")
                        for s5 in range(MS)
                    ]
                for d in range(3):
                    v2p = psum.tile([128, C], f32, name=f"v2p{H}{blk}{i}{d}", tag="mm")
                    for s5 in range(NS):
                        cs = slice(s5 * 512, min((s5 + 1) * 512, C))
                        nc.tensor.matmul(v2p[:, cs], Wv[blk][:], vsrc[:, d, sl][:, cs],
                                         start=True, stop=True)
                    if blk == 0:
                        nc.vector.tensor_tensor(vnext[:, d, sl], v2p[:], gate[:], ALU.mult)
                    else:
                        vx = sxpool.tile([128, C], f32, name=f"vx{H}{i}{d}", tag="sx")
                        nc.vector.tensor_tensor(vx[:], v2p[:], gate[:], ALU.mult)
                        for s5 in range(MS):
                            cs = slice(s5 * MC, (s5 + 1) * MC)
                            nc.tensor.matmul(metp[s5][:], WVFC[d][:], vx[:, cs],
                                             start=(d == 0), stop=False)
                if blk == 1:
                    for s5 in range(MS):
                        cs = slice(s5 * MC, (s5 + 1) * MC)
                        nc.tensor.matmul(metp[s5][:], wt["WsC3"][:], s2t[:, cs],
                                         start=False, stop=True)
                        mets = metsb.tile([4, MC], f32, name=f"mets{H}{i}{s5}", tag="met_sb")
                        nc.scalar.copy(mets[:], metp[s5][:])
                        nc.sync.dma_start(
                            met_out[:, h0 + i * C + s5 * MC:h0 + i * C + (s5 + 1) * MC],
                            mets[:],
                        )

    for rep in range(reps):
        _body(rep)
    ctx.close()


def _prep_weights(inputs):
    """Host-side: split / pad the small weights into lhsT-ready arrays."""
    g = lambda k: np.ascontiguousarray(np.asarray(inputs[k], dtype=np.float32))
    out = {}
    for blk in range(2):
        out[f"Wu{blk}"] = g(f"Wu{blk}")
        out[f"Wv{blk}"] = g(f"Wv{blk}")
        W1 = g(f"W1_{blk}")
        out[f"W1a{blk}"] = np.ascontiguousarray(W1[:128, :])
        out[f"W1b{blk}"] = np.ascontiguousarray(W1[128:, :])
        W2 = g(f"W2_{blk}")
        out[f"W2a{blk}"] = np.ascontiguousarray(W2[:, :128])
        out[f"W2b{blk}"] = np.ascontiguousarray(W2[:, 128:])
        b1 = g(f"b1_{blk}").reshape(128, 1)
        b2 = g(f"b2_{blk}")
        out[f"b1_{blk}"] = np.ascontiguousarray(b1)
        out[f"b2a{blk}"] = np.ascontiguousarray(b2[:128].reshape(128, 1))
        out[f"b2b{blk}"] = np.ascontiguousarray(b2[128:].reshape(128, 1))
    Wvf = g("Wvf").reshape(128)
    Ws = g("Ws").reshape(128)
    for d in range(3):
        c = np.zeros((128, 4), np.float32)
        c[:, d] = Wvf
        out[f"WvfC{d}"] = c
    c = np.zeros((128, 4), np.float32)
    c[:, 3] = Ws
    out["WsC3"] = c
    return out


_CACHE = {}


def _build_program(m, reps=1):
    if (m, reps) in _CACHE:
        return _CACHE[(m, reps)]
    from contextlib import ExitStack
    import concourse.bacc as bacc
    import concourse.mybir as mybir
    from concourse.tile import TileContext

    f32 = mybir.dt.float32
    nc = bacc.Bacc("TRN2", target_bir_lowering=False, debug=False, num_devices=1)
    ins = {}
    ins["sT"] = nc.dram_tensor("sT", [128, m], f32, kind="ExternalInput").ap()
    ins["vT"] = nc.dram_tensor("vT", [128, 3, m], f32, kind="ExternalInput").ap()
    for name in ("Wu0", "Wv0", "Wu1", "Wv1", "W1a0", "W1b0", "W1a1", "W1b1",
                 "W2a0", "W2b0", "W2a1", "W2b1"):
        ins[name] = nc.dram_tensor(name, [128, 128], f32, kind="ExternalInput").ap()
    for name in ("WvfC0", "WvfC1", "WvfC2", "WsC3"):
        ins[name] = nc.dram_tensor(name, [128, 4], f32, kind="ExternalInput").ap()
    for name in ("b1_0", "b1_1", "b2a0", "b2b0", "b2a1", "b2b1"):
        ins[name] = nc.dram_tensor(name, [128, 1], f32, kind="ExternalInput").ap()
    outs = {"met": nc.dram_tensor("met", [4, m], f32, kind="ExternalOutput").ap()}

    with TileContext(nc) as tc:
        _emit(tc, outs, ins, m, reps=reps)
    nc.finalize()
    _CACHE[(m, reps)] = nc
    return nc


def _host_combine(met_full, z0, r0, mask2d):
    """met_full: [4, N] per-atom metrics; final reduction in float64."""
    vv = met_full[:3, :].T.astype(np.float64)          # [N, 3]
    s_out = met_full[3, :].astype(np.float64)          # [N]
    m64 = mask2d.astype(np.float64)                    # [NB, N]
    z64 = z0.astype(np.float64)
    r64 = r0.astype(np.float64)

    A = m64 @ vv                                       # [NB, 3]
    B = m64 @ (s_out[:, None] * r64)                   # [NB, 3]
    Nz = m64 @ (z64[:, None] * r64)                    # [NB, 3]
    Dz = m64 @ z64                                     # [NB]
    S = s_out.sum()
    centre = Nz / Dz[:, None]
    mu = A + B - centre * S
    return np.linalg.norm(mu, axis=-1, keepdims=True).astype(np.float32)  # [NB,1]


def kernel(**inputs):
    from concourse.bass_utils import run_bass_kernel_spmd

    z = np.asarray(inputs["z"], dtype=np.float32)
    s = np.asarray(inputs["s"], dtype=np.float32)
    v = np.asarray(inputs["v"], dtype=np.float32)
    r = np.asarray(inputs["r"], dtype=np.float32)
    batch_mask = np.asarray(inputs["batch_mask"], dtype=np.float32)

    w = _prep_weights(inputs)

    sT_full = np.ascontiguousarray(s[0].T)                     # [128, N]
    vT_full = np.ascontiguousarray(v[0].transpose(2, 1, 0))    # [128, 3, N]

    in_maps = []
    for c in range(NCORES):
        a, b = c * M, (c + 1) * M
        im = dict(w)
        im["sT"] = np.ascontiguousarray(sT_full[:, a:b])
        im["vT"] = np.ascontiguousarray(vT_full[:, :, a:b])
        in_maps.append(im)

    nc = _build_program(M)
    res = run_bass_kernel_spmd(nc, in_maps, core_ids=list(range(NCORES)))
    met_full = np.concatenate([res.results[c]["met"] for c in range(NCORES)], axis=1)

    return _host_combine(met_full, z[0], r[0], batch_mask[:, :, 0])
